# revision 1
# baseline (speedup 1.0000x reference)
"""Trainium2 Bass kernel for nn_CrossAttention (B=4, S=1024, D=512, H=8).

Sharding: 8 cores = batch (4) x head-group (2 groups of 4 heads).
Each core computes a partial [S, E] output over its 256 feature dims;
the host sums the two partials per batch and adds the bias.

Per-core math (feature-major / transposed activation layout):
  allT  [512, 2049] = [l2rT | r2lT | tembT]           (k order: l2r, r2l, temb)
  kvT   [256, 2049] = Wk_slice @ allT                 (shared q/k/v projection)
  qT    [256, 1024] = kvT[:, :1024] + kvT[:, 1024:2048]   (linearity of proj)
  per head h (hd=64), per 512-query tile, per visible 128-k block:
    logitsT [128k, 512q] = kvT_h_blk.T-contract @ qT_h    (PE, K=64)
    expT = exp(0.125 * logitsT)                           (ACT, reads PSUM)
    triangular/pad masks applied with in-place affine_select(fill=0)
    xT [65, 512] += kv_aug_blk.T-contract @ expT          (kv_aug has ones col
                                                           -> row 64 = denom)
  normalize via reciprocal + PE ones-broadcast, then
  out_part [1024, 512] = xT.T @ Wo_slice  (accumulated over 4 heads)
"""

import sys

sys.path.insert(0, "/opt/trn_rl_repo")

from contextlib import ExitStack

import numpy as np

import concourse.bass as bass
import concourse.mybir as mybir
import concourse.tile as tile
from concourse import bacc
from concourse.bass import ds, ts
from concourse.bass_utils import run_bass_kernel_spmd
from concourse.masks import make_identity


def _ensure_ntff_hook():
    """This image's antenv lacks axon_hooks; synthesize it so trace=True can
    reach the libaxon NTFF profiler (used by test.py, harmless otherwise)."""
    import types

    try:
        from antenv import axon_hooks  # noqa: F401

        return
    except ImportError:
        pass
    mod = types.ModuleType("antenv.axon_hooks")
    mod._hook = None
    mod.set_axon_ntff_profile_hook = lambda h: setattr(mod, "_hook", h)
    mod.get_axon_ntff_profile_hook = lambda: mod._hook
    import antenv

    sys.modules["antenv.axon_hooks"] = mod
    antenv.axon_hooks = mod
    try:
        from trn_agent_boot.trn_boot import _ntff_profile_via_ctypes

        mod._hook = _ntff_profile_via_ctypes("/opt/axon/libaxon_pjrt.so")
    except Exception:
        pass


_ensure_ntff_hook()


def _enable_ldw_opt():
    """Flip walrus's --enable-ldw-opt (hardcoded false in bass_utils): with
    one LDWEIGHTS per matmul serialized against its MM, ~80us of the PE span
    is weight loads. Opt-out via KERNEL_LDW_OPT=0."""
    import os

    if os.environ.get("KERNEL_LDW_OPT", "0") != "1":
        return
    import concourse.bass_utils as _bu

    orig = _bu.run_command

    def patched(argv, **kwargs):
        argv = [
            "--enable-ldw-opt=true" if a == "--enable-ldw-opt=false" else a
            for a in argv
        ]
        return orig(argv, **kwargs)

    if getattr(_bu.run_command, "_ldw_patched", None) is None:
        patched._ldw_patched = True
        _bu.run_command = patched


_enable_ldw_opt()

F32 = mybir.dt.float32
F32R = mybir.dt.float32r
BF16 = mybir.dt.bfloat16
AF = mybir.ActivationFunctionType
ALU = mybir.AluOpType

P = 128
S = 1024
D = 512
E = 512
HG = 4  # heads per core
HD = 64
CS = HG * HD  # 256 feature cols per core
NKB = 17  # padded k blocks: 8 l2r + 8 r2l + 1 (temb+pad)
KL = NKB * P  # 2176
KV_REAL = 2 * S + 1  # 2049


def _r(ap):
    return ap.bitcast(F32R)


def _visible_blocks(q0):
    """k blocks visible to query tile [q0, q0+512); (kb, mask) with mask None
    for fully-visible, else (kind, d)."""
    vis = []
    for kb in range(8):  # l2r keys: visible iff q >= j
        d = 128 * kb - q0
        if d >= 512:
            continue  # fully masked
        vis.append((kb, None if d <= -128 else ("l2r", d)))
    for kbl in range(8):  # r2l keys: visible iff q <= j
        d = 128 * kbl - q0
        if d <= -128:
            continue  # fully masked
        vis.append((8 + kbl, None if d >= 511 else ("r2l", d)))
    vis.append((16, ("temb", 0)))  # partition 0 = temb col, rest = pad
    return vis


def _build_body(ctx, tc):
    nc = tc.nc
    ctx.enter_context(
        nc.allow_low_precision(reason="f32r rounding discipline for PE matmuls")
    )

    xlT = nc.dram_tensor("xlT", [D, S], F32, kind="ExternalInput").ap()
    xrT = nc.dram_tensor("xrT", [D, S], F32, kind="ExternalInput").ap()
    tembT = nc.dram_tensor("tembT", [D, 1], F32, kind="ExternalInput").ap()
    wkT = nc.dram_tensor("wkT", [D, CS], F32, kind="ExternalInput").ap()
    woT = nc.dram_tensor("woT", [CS, E], F32, kind="ExternalInput").ap()
    out = nc.dram_tensor("out_part", [S, E], F32, kind="ExternalOutput").ap()

    const = ctx.enter_context(tc.tile_pool(name="const", bufs=1))
    inp = ctx.enter_context(tc.tile_pool(name="inp", bufs=1))
    kvp = ctx.enter_context(tc.tile_pool(name="kvp", bufs=1))
    kvag = ctx.enter_context(tc.tile_pool(name="kvag", bufs=1))
    xts = ctx.enter_context(tc.tile_pool(name="xts", bufs=1))
    expp = ctx.enter_context(tc.tile_pool(name="expp", bufs=30))
    sres = ctx.enter_context(tc.tile_pool(name="sres", bufs=4))
    outp = ctx.enter_context(tc.tile_pool(name="outp", bufs=3))
    ps512 = ctx.enter_context(tc.tile_pool(name="ps512", bufs=3, space="PSUM"))
    psX = ctx.enter_context(tc.tile_pool(name="psX", bufs=4, space="PSUM"))
    psC = ctx.enter_context(tc.tile_pool(name="psC", bufs=1, space="PSUM"))

    ident = const.tile([P, P], BF16)
    ident_stage = const.tile([P, P], F32)
    make_identity(nc, ident_stage[:])
    nc.vector.tensor_copy(ident[:], ident_stage[:])  # cast to bf16
    ones = const.tile([65, HD], F32)
    ones_stage = const.tile([65, HD], F32)
    nc.gpsimd.memset(ones_stage[:], 1.0)
    nc.vector.tensor_copy(_r(ones[:]), ones_stage[:])  # round to f32r

    # ---- input DMAs ----
    # f32r matmul operands must be *written* by a rounding compute op, so DMA
    # lands in staging tiles and ACT/DVE round-copies into the real tiles.
    stg = ctx.enter_context(tc.tile_pool(name="stg", bufs=4))
    allT = [inp.tile([P, KV_REAL + 1], F32, name=f"allT{j}") for j in range(4)]
    for j in range(4):
        for src_ap, c0, w in (
            (xlT[ts(j, P), :], 0, S),
            (xrT[ts(j, P), :], S, S),
            (tembT[ts(j, P), :], 2 * S, 1),
        ):
            st = stg.tile([P, 1024], F32, name="st", tag="st")
            nc.sync.dma_start(out=st[:, 0:w], in_=src_ap)
            if w == 1:  # fp32r matmul needs even free counts: add a zero col
                nc.vector.memset(st[:, 1:2], 0.0)
                w = 2
            nc.scalar.activation(_r(allT[j][:, ds(c0, w)]), st[:, 0:w], AF.Copy)
    wk = inp.tile([P, 4, CS], F32)
    st = stg.tile([P, 1024], F32, name="st", tag="st")
    nc.sync.dma_start(
        out=st[:].rearrange("p (c n) -> p c n", c=4),
        in_=wkT.rearrange("(c p) n -> p c n", p=P),
    )
    nc.vector.tensor_copy(_r(wk[:]), st[:].rearrange("p (c n) -> p c n", c=4))
    wo = inp.tile([HD, HG, E], F32)
    for half in range(2):
        st = stg.tile([P, 1024], F32, name="st", tag="st")
        nc.sync.dma_start(
            out=st[0:HD, :].rearrange("p (c n) -> p c n", c=2),
            in_=woT.rearrange("(g p) n -> p g n", p=HD)[:, ds(half * 2, 2), :],
        )
        nc.vector.tensor_copy(
            _r(wo[:, ds(half * 2, 2), :]),
            st[0:HD, :].rearrange("p (c n) -> p c n", c=2),
        )

    # ---- shared qkv projection: kvT[c][128, KL], c-chunks of 128 ----
    kvT = [kvp.tile([P, KL], BF16, name=f"kvT{c}") for c in range(2)]
    qT = [kvp.tile([P, S], BF16, name=f"qT{c}") for c in range(2)]
    ntiles = [(0, 512), (512, 512), (1024, 512), (1536, 512), (2048, 2)]
    zst = stg.tile([P, 1024], F32, name="zst", tag="st")
    nc.vector.memset(zst[:], 0.0)
    for c in range(2):
        # zero pad cols via rounded copy (f32r memset fails ISA check)
        nc.vector.tensor_copy(kvT[c][:, KV_REAL:KL], zst[:, 0 : KL - KV_REAL])
        for n0, nw in ntiles:
            pp = ps512.tile([P, 512], F32, name="pp", tag="ps")
            for j in range(4):
                nc.tensor.matmul(
                    pp[:, 0:nw],
                    _r(wk[:, j, ts(c, P)]),
                    _r(allT[j][:, ds(n0, nw)]),
                    start=(j == 0),
                    stop=(j == 3),
                )
            nc.vector.tensor_copy(kvT[c][:, ds(n0, nw)], pp[:, 0:nw])
        nc.vector.tensor_add(qT[c][:], kvT[c][:, 0:S], kvT[c][:, S : 2 * S])

    # ---- kv in natural [k, d] layout, ones-augmented: kva[h][128, 17, 65] ----
    kva = [kvag.tile([P, NKB, 65], BF16, name=f"kva{h}") for h in range(HG)]
    ost = stg.tile([P, NKB], F32, name="ost", tag="st")
    nc.vector.memset(ost[:], 1.0)
    for h in range(HG):
        c, ho = h // 2, 64 * (h % 2)
        # only the ones-columns need init; transposes fill cols 0..63
        nc.vector.tensor_copy(
            kva[h][:, :, 64:65],
            ost[:].rearrange("p (a b) -> p a b", b=1),
        )
        for g, nblk in ((0, 8), (1, 8), (2, 1)):
            tp = ps512.tile([P, 8, HD], BF16, name="tp", tag="ps")
            for b in range(nblk):
                kb = g * 8 + b
                nc.tensor.transpose(
                    tp[:, b, :],
                    kvT[c][ho : ho + HD, ts(kb, P)],
                    ident[ho : ho + HD, ho : ho + HD],
                )
            nc.vector.tensor_copy(
                kva[h][:, ds(g * 8, nblk), 0:HD], tp[:, 0:nblk, :]
            )

    # ---- precomputed 0/1 bf16 mask tiles (DVE mul is ~7x cheaper than a
    # per-block gpsimd affine_select) ----
    maskp = ctx.enter_context(tc.tile_pool(name="maskp", bufs=1))
    ones_bf = maskp.tile([P, 512], BF16)
    nc.gpsimd.memset(ones_bf[:], 1.0)
    masks = {}
    for d in (0, 128, 256, 384):
        mt = maskp.tile([P, 512], BF16, name=f"ml2r{d}")
        nc.gpsimd.affine_select(
            mt[:], ones_bf[:], pattern=[[1, 512]], compare_op=ALU.is_ge,
            fill=0.0, base=-d, channel_multiplier=-1,
        )
        masks[("l2r", d)] = mt
        mt = maskp.tile([P, 512], BF16, name=f"mr2l{d}")
        nc.gpsimd.affine_select(
            mt[:], ones_bf[:], pattern=[[-1, 512]], compare_op=ALU.is_ge,
            fill=0.0, base=d, channel_multiplier=1,
        )
        masks[("r2l", d)] = mt
    mt = maskp.tile([P, 512], BF16, name="mtemb")
    nc.gpsimd.affine_select(
        mt[:], ones_bf[:], pattern=[[0, 512]], compare_op=ALU.is_ge,
        fill=0.0, base=0, channel_multiplier=-1,
    )
    masks[("temb", 0)] = mt

    # ---- attention ----
    # Emit all logits MMs of a stream before its AV MMs: PE is in-order, so
    # interleaving lg/av stalls PE on the ACT exp + gpsimd mask chain (and the
    # stalls keep HAM cold, halving the PE clock).
    xt = [xts.tile([HD, S], F32, name=f"xt{h}") for h in range(HG)]
    for c in range(2):
        hpair = (2 * c, 2 * c + 1)
        for qi in range(2):
            q0 = qi * 512
            vis = _visible_blocks(q0)
            xps = {h: psX.tile([65, 512], F32, name=f"xps{h % 2}", tag="xps") for h in hpair}
            exs = []
            for kb, mask in vis:
                exh = {}
                for h in hpair:
                    ho = 64 * (h % 2)
                    lg = ps512.tile([P, 512], F32, name="lg", tag="ps")
                    nc.tensor.matmul(
                        lg[:],
                        kvT[c][ho : ho + HD, ts(kb, P)],
                        qT[c][ho : ho + HD, ds(q0, 512)],
                        start=True,
                        stop=True,
                    )
                    ex = expp.tile([P, 512], BF16, name="ex")
                    nc.scalar.activation(ex[:], lg[:], AF.Exp, scale=0.125)
                    if mask is not None:
                        nc.vector.tensor_mul(ex[:], ex[:], masks[mask][:])
                    exh[h] = ex
                exs.append((kb, exh))
            for i, (kb, exh) in enumerate(exs):
                for h in hpair:
                    nc.tensor.matmul(
                        xps[h][:],
                        kva[h][:, kb, :],
                        exh[h][:],
                        start=(i == 0),
                        stop=(i == len(exs) - 1),
                    )
            for h in hpair:
                cs = sres.tile([65, 512], F32, name="cs")
                nc.vector.tensor_copy(_r(cs[64:65, :]), xps[h][64:65, :])
                bc = psC.tile([HD, 512], F32, name="bc")
                nc.tensor.matmul(
                    bc[:], _r(ones[64:65, :]), _r(cs[64:65, :]),
                    start=True, stop=True,
                )
                bcs = sres.tile([HD, 512], F32, name="bcs")
                nc.vector.reciprocal_approx_fast(bcs[:], bc[:])
                nc.vector.tensor_mul(
                    _r(xt[h][:, ds(q0, 512)]), xps[h][0:HD, :], bcs[:]
                )

    # ---- output projection: out[s, e] += xt[h].T @ wo_h ----
    for st in range(8):
        pf = ps512.tile([P, E], F32, name="pf", tag="ps")
        for h in range(HG):
            nc.tensor.matmul(
                pf[:],
                _r(xt[h][:, ts(st, P)]),
                _r(wo[:, h, :]),
                start=(h == 0),
                stop=(h == 3),
            )
        ob = outp.tile([P, E], F32, name="ob")
        nc.vector.tensor_copy(ob[:], pf[:])
        nc.sync.dma_start(out=out[ts(st, P), :], in_=ob[:])


_NC_CACHE = None


def build_nc():
    global _NC_CACHE
    if _NC_CACHE is None:
        nc = bacc.Bacc(
            "TRN2",
            target_bir_lowering=False,
            debug=False,
            num_devices=8,
        )
        with tile.TileContext(nc) as tc, ExitStack() as ctx:
            _build_body(ctx, tc)
        nc.compile()
        _NC_CACHE = nc
    return _NC_CACHE


def make_in_maps(l2r_embed, r2l_embed, temb, W_dense, W_out):
    in_maps = []
    for core in range(8):
        b, hg = core // 2, core % 2
        cols = slice(CS * hg, CS * (hg + 1))
        in_maps.append(
            {
                "xlT": np.ascontiguousarray(l2r_embed[b].T),
                "xrT": np.ascontiguousarray(r2l_embed[b].T),
                "tembT": np.ascontiguousarray(temb[b][:, None]),
                "wkT": np.ascontiguousarray(W_dense[cols, :].T),
                "woT": np.ascontiguousarray(W_out[:, cols].T),
            }
        )
    return in_maps


def kernel(l2r_embed, r2l_embed, temb, W_dense, W_out, b_out, num_heads, **run_kwargs):
    assert int(num_heads) == 8
    l2r_embed = np.asarray(l2r_embed, np.float32)
    r2l_embed = np.asarray(r2l_embed, np.float32)
    temb = np.asarray(temb, np.float32)
    W_dense = np.asarray(W_dense, np.float32)
    W_out = np.asarray(W_out, np.float32)
    b_out = np.asarray(b_out, np.float32)

    nc = build_nc()
    in_maps = make_in_maps(l2r_embed, r2l_embed, temb, W_dense, W_out)
    res = run_bass_kernel_spmd(nc, in_maps, core_ids=list(range(8)), **run_kwargs)

    B = l2r_embed.shape[0]
    outp = np.empty((B, S, E), np.float32)
    for b in range(B):
        outp[b] = (
            res.results[2 * b]["out_part"]
            + res.results[2 * b + 1]["out_part"]
            + b_out[None, :]
        )
    if run_kwargs:
        kernel.last_results = res
    return outp



# revision 17
# speedup vs baseline: 1.2701x; 1.2701x over previous
"""Trainium2 Bass kernel for nn_CrossAttention (B=4, S=1024, D=512, H=8).

Sharding: 8 cores = batch (4) x head-group (2 groups of 4 heads).
Each core computes a partial [S, E] output over its 256 feature dims;
the host sums the two partials per batch and adds the bias.

v2 design notes (per core; heads h=0..3, c=h//2, ho=64*(h%2)):
  - inputs shipped fp16 (halves DMA bytes); proj j-outer so PE overlaps DMA
  - kvT[c] [128f, 2048k] bf16 (l2r keys 0..1023, r2l keys 1024..2047),
    temb key kept separately; qT = kvT_l2r + kvT_r2l (proj linearity)
  - logits per (h, kb): key block kb is visible to a CONTIGUOUS q range
    (l2r kb: q in [128kb, 1024); r2l kb: q in [0, 128(kb-8)+128)), so the
    matmuls and the single big exp are trimmed to it; only the diagonal
    128x128 needs a triangular mask (gpsimd affine_select in place)
  - temb logits for all 4 heads in one col-tiled concurrent MM group;
    temb AV is a rank-1 update that doubles as the PSUM initializer
    (full-width start=True)
  - AV: xp[65, 1024] per head via ones-augmented kva (row 64 = denom);
    AV lags logits by 2 blocks so PE isn't blocked on ACT exp
  - normalize batched per head: one [1,1024] reciprocal, PE ones-broadcast
    (f32r rhs), DVE mul
  - out projection packs the 2 heads of a c-pair into one 128-contract MM
  - PSUM: psA 2x[128,1024] (proj/logits/bc/outproj) + psX 2x[65,1024] = 8 banks
"""

import sys

sys.path.insert(0, "/opt/trn_rl_repo")

from contextlib import ExitStack

import numpy as np

import concourse.bass as bass
import concourse.mybir as mybir
import concourse.tile as tile
from concourse import bacc
from concourse.bass import ds, ts
from concourse.bass_utils import run_bass_kernel_spmd
from concourse.masks import make_identity


def _ensure_ntff_hook():
    """This image's antenv lacks axon_hooks; synthesize it so trace=True can
    reach the libaxon NTFF profiler (used by test.py, harmless otherwise)."""
    import types

    try:
        from antenv import axon_hooks  # noqa: F401

        return
    except ImportError:
        pass
    mod = types.ModuleType("antenv.axon_hooks")
    mod._hook = None
    mod.set_axon_ntff_profile_hook = lambda h: setattr(mod, "_hook", h)
    mod.get_axon_ntff_profile_hook = lambda: mod._hook
    import antenv

    sys.modules["antenv.axon_hooks"] = mod
    antenv.axon_hooks = mod
    try:
        from trn_agent_boot.trn_boot import _ntff_profile_via_ctypes

        mod._hook = _ntff_profile_via_ctypes("/opt/axon/libaxon_pjrt.so")
    except Exception:
        pass


_ensure_ntff_hook()

F32 = mybir.dt.float32
F32R = mybir.dt.float32r
F16 = mybir.dt.float16
BF16 = mybir.dt.bfloat16
AF = mybir.ActivationFunctionType
ALU = mybir.AluOpType

P = 128
S = 1024
D = 512
E = 512
HG = 4  # heads per core
HD = 64
CS = HG * HD  # 256 feature cols per core
NKB = 16  # 8 l2r + 8 r2l key blocks (temb handled separately)


def _r(ap):
    return ap.bitcast(F32R)


def _vis(kb):
    """Global q range visible for key block kb (l2r kb<8, r2l kb>=8)."""
    if kb < 8:
        return (128 * kb, 1024)
    return (0, 128 * (kb - 8) + 128)


def _chunks(s, e):
    """Split [s, e) at the 512 boundary (PSUM bank / moving-dim limit)."""
    out = []
    if s < 512:
        out.append((s, min(e, 512)))
    if e > 512:
        out.append((max(s, 512), e))
    return out


def _build_body(ctx, tc):
    nc = tc.nc
    ctx.enter_context(
        nc.allow_low_precision(reason="bf16/fp16 matmul discipline")
    )

    xlT = nc.dram_tensor("xlT", [D, S], F16, kind="ExternalInput").ap()
    xrT = nc.dram_tensor("xrT", [D, S], F16, kind="ExternalInput").ap()
    tembT = nc.dram_tensor("tembT", [D, 2], F16, kind="ExternalInput").ap()
    wkT = nc.dram_tensor("wkT", [D, CS], F16, kind="ExternalInput").ap()
    woT = nc.dram_tensor("woT", [CS, E], BF16, kind="ExternalInput").ap()
    out = nc.dram_tensor("out_part", [S, E], F32, kind="ExternalOutput").ap()

    const = ctx.enter_context(tc.tile_pool(name="const", bufs=1))
    inp = ctx.enter_context(tc.tile_pool(name="inp", bufs=1))
    kvp = ctx.enter_context(tc.tile_pool(name="kvp", bufs=1))
    expp = ctx.enter_context(tc.tile_pool(name="expp", bufs=4))
    xts = ctx.enter_context(tc.tile_pool(name="xts", bufs=1))
    outp = ctx.enter_context(tc.tile_pool(name="outp", bufs=2))
    # PSUM: big 2-bank tiles (proj / logits / temb-logits / bc / out-proj)
    psA = ctx.enter_context(tc.tile_pool(name="psA", bufs=2, space="PSUM"))
    # PSUM: [65, 1024] per-head attention accumulators
    psX = ctx.enter_context(tc.tile_pool(name="psX", bufs=2, space="PSUM"))

    ident = const.tile([P, P], BF16)
    ident_stage = const.tile([P, P], F32)
    make_identity(nc, ident_stage[:])
    nc.vector.tensor_copy(ident[:], ident_stage[:])
    ones_bc = const.tile([P, HD], BF16)
    nc.gpsimd.memset(ones_bc[:], 1.0)

    # ---- input DMAs (fp16; temb slivers first so temb proj starts early) ----
    wk = inp.tile([P, 4, CS], F16)
    nc.sync.dma_start(out=wk[:], in_=wkT.rearrange("(c p) n -> p c n", p=P))
    allT = [inp.tile([P, 2 * S + 2], F16, name=f"allT{j}") for j in range(4)]
    for j in range(4):
        nc.sync.dma_start(out=allT[j][:, 2 * S : 2 * S + 2], in_=tembT[ts(j, P), :])
    for j in range(4):
        nc.sync.dma_start(out=allT[j][:, 0:S], in_=xlT[ts(j, P), :])
        nc.sync.dma_start(out=allT[j][:, S : 2 * S], in_=xrT[ts(j, P), :])
    wo = inp.tile([P, 2, E], BF16)
    nc.sync.dma_start(out=wo[:], in_=woT.rearrange("(c p) n -> p c n", p=P))

    # ---- temb projection (tiny, runs first): kvtb[c][128f, 2] ----
    kvtb = [kvp.tile([P, 2], BF16, name=f"kvtb{c}") for c in range(2)]
    tas = []
    for c in range(2):
        ta = psA.tile([P, 2 * 512], F32, name="ta", tag="psA")
        for j in range(4):
            nc.tensor.matmul(
                ta[:, 0:2], wk[:, j, ts(c, P)], allT[j][:, ds(2 * S, 2)],
                start=(j == 0), stop=(j == 3),
            )
        tas.append(ta)
    for c in range(2):
        nc.vector.tensor_copy(kvtb[c][:], tas[c][:, 0:2])

    # ---- shared qkv projection: kvT[c][128f, 2048k] bf16 ----
    kvT = [kvp.tile([P, 2 * S], BF16, name=f"kvT{c}") for c in range(2)]
    qT = [kvp.tile([P, S], BF16, name=f"qT{c}") for c in range(2)]
    for c in range(2):
        pa = psA.tile([P, 2 * 512], F32, name="pa", tag="psA")
        pb = psA.tile([P, 2 * 512], F32, name="pb", tag="psA")
        for j in range(4):
            lw = wk[:, j, ts(c, P)]
            for n in range(4):
                dst = (pa if n < 2 else pb)[:, ds(512 * (n % 2), 512)]
                nc.tensor.matmul(
                    dst, lw, allT[j][:, ds(512 * n, 512)],
                    start=(j == 0), stop=(j == 3),
                )
        nc.vector.tensor_copy(kvT[c][:, 0:1024], pa[:])
        nc.vector.tensor_copy(kvT[c][:, 1024:2048], pb[:])
        nc.vector.tensor_add(qT[c][:], kvT[c][:, 0:S], kvT[c][:, S : 2 * S])

    # ---- kv in k-major layout for AV: kva[c][128k, 16kb, 130] ----
    # cols 0:64 = head 2c feats, 64 = ones, 65:129 = head 2c+1 feats, 129 = ones
    kva = [kvp.tile([P, NKB, 130], BF16, name=f"kva{c}") for c in range(2)]
    for c in range(2):
        nc.gpsimd.memset(kva[c][:, :, 64:65], 1.0)
        nc.gpsimd.memset(kva[c][:, :, 129:130], 1.0)
        for g in range(2):  # 8 key blocks per staging tile
            tp = psA.tile([P, 8 * P], BF16, name="tp", tag="psA")
            for b in range(8):
                nc.tensor.transpose(
                    tp[:, ds(128 * b, P)],
                    kvT[c][:, ts(8 * g + b, P)],
                    ident[:],
                )
            nc.vector.tensor_copy(
                kva[c][:, ds(8 * g, 8), :]
                .rearrange("p b (g2 x) -> p b g2 x", g2=2)[:, :, :, 0:HD],
                tp[:].rearrange("p (b g2 x) -> p b g2 x", b=8, g2=2),
            )

    # kvta[128, 65]: row 32h = [temb-key feats of head h (64), 1.0]
    kvta = kvp.tile([P, HD + 1], BF16, name="kvta")
    nc.vector.memset(kvta[:, HD : HD + 1], 1.0)
    tpt = psA.tile([P, 2 * P], BF16, name="tpt", tag="psA")
    for c in range(2):
        nc.tensor.transpose(tpt[0:2, ds(128 * c, P)], kvtb[c][:], ident[:])
    for h in range(HG):
        c, hp = h // 2, h % 2
        nc.vector.tensor_copy(
            kvta[ds(32 * h, 1), 0:HD],
            tpt[0:1, ds(128 * c + 64 * hp, HD)],
        )

    # ---- temb logits for all 4 heads (col-tiled, concurrent) + exp ----
    TE = kvp.tile([P, S], BF16, name="TE")  # row 32h = exp temb logits head h
    tl = psA.tile([P, 2 * 512], F32, name="tl", tag="psA")
    nc.vector.memset(tl[:], 0.0)
    for h in range(HG):
        c, ho = h // 2, 64 * (h % 2)
        for qi in range(2):
            nc.tensor.matmul(
                tl[ds(32 * h, 1), ds(512 * qi, 512)],
                kvtb[c][ds(ho, HD), 0:1],
                qT[c][ds(ho, HD), ds(512 * qi, 512)],
                start=True, stop=True,
                tile_position=(ho, 32 * h),
            )
    nc.scalar.activation(TE[:], tl[:], AF.Exp, scale=0.125)

    # ---- attention: per head, per key block; AV lags logits by 2 blocks ----
    xt2 = [xts.tile([P, S], BF16, name=f"xt2{c}") for c in range(2)]
    rec = kvp.tile([P, S], BF16, name="rec")
    xpool = ctx.enter_context(tc.tile_pool(name="xpool", bufs=2))

    def emit_lg(h, kb):
        c, ho = h // 2, 64 * (h % 2)
        s, e = _vis(kb)
        lg = psA.tile([P, 2 * 512], F32, name="lg", tag="psA")
        for cs_, ce in _chunks(s, e):
            nc.tensor.matmul(
                lg[:, ds(cs_, ce - cs_)],
                kvT[c][ds(ho, HD), ts(kb, P)],
                qT[c][ds(ho, HD), ds(cs_, ce - cs_)],
                start=True, stop=True,
            )
        ex = expp.tile([P, S], BF16, name="ex")
        nc.scalar.activation(
            ex[:, ds(s, e - s)], lg[:, ds(s, e - s)], AF.Exp, scale=0.125
        )
        if kb < 8:  # diag: keep q >= k  (iota = f - p >= 0)
            dcol = 128 * kb
            nc.gpsimd.affine_select(
                ex[:, ds(dcol, P)], ex[:, ds(dcol, P)],
                pattern=[[1, P]], compare_op=ALU.is_ge,
                fill=0.0, base=0, channel_multiplier=-1,
            )
        else:  # diag: keep q <= k  (iota = p - f >= 0)
            dcol = e - 128
            nc.gpsimd.affine_select(
                ex[:, ds(dcol, P)], ex[:, ds(dcol, P)],
                pattern=[[-1, P]], compare_op=ALU.is_ge,
                fill=0.0, base=0, channel_multiplier=1,
            )
        return (kb, s, e, ex)

    def emit_av(h, xp, item):
        c = h // 2
        kb, s, e, ex = item
        for cs_, ce in _chunks(s, e):
            nc.tensor.matmul(
                xp[:, ds(cs_, ce - cs_)],
                kva[c][:, kb, ds(65 * (h % 2), 65)],
                ex[:, ds(cs_, ce - cs_)],
                start=False, stop=(kb == 15),
                skip_group_check=True,
            )

    for h in range(HG):
        c, ho = h // 2, 64 * (h % 2)
        xp = psX.tile([65, 2 * 512], F32, name="xp", tag="psX")
        # rank-1 temb AV doubles as the full-width PSUM initializer
        for qi in range(2):
            nc.tensor.matmul(
                xp[:, ds(512 * qi, 512)],
                kvta[ds(32 * h, 1), :],
                TE[ds(32 * h, 1), ds(512 * qi, 512)],
                start=True, stop=False,
                tile_position=(32 * h, 0),
                skip_group_check=True,
            )
        pend = []
        for kb in range(NKB):
            pend.append(emit_lg(h, kb))
            if len(pend) > 2:
                emit_av(h, xp, pend.pop(0))
        for item in pend:
            emit_av(h, xp, item)
        # normalize: one reciprocal per head, PE ones-broadcast, DVE mul
        import os as _os

        if _os.environ.get("KDBG", "0") == "1":
            xps = xpool.tile([65, 2 * 512], F32, name="xps")
            nc.vector.tensor_copy(xps[:], xp[:])
            dxp = nc.dram_tensor(
                f"dbg_xps{h}", [65, 2 * 512], F32, kind="ExternalOutput"
            ).ap()
            nc.sync.dma_start(out=dxp, in_=xps[:])
        # baseline-proven order: copy denom row, PE-broadcast it, then
        # reciprocal on the [64, 1024] broadcast, then normalize-mul
        nc.vector.tensor_copy(rec[ds(32 * h, 1), :], xp[64:65, :])
        bc = psA.tile([P, 2 * 512], F32, name="bc", tag="psA")
        for qi in range(2):
            nc.tensor.matmul(
                bc[0:HD, ds(512 * qi, 512)],
                ones_bc[ds(32 * h, 1), :],
                rec[ds(32 * h, 1), ds(512 * qi, 512)],
                start=True, stop=True,
                tile_position=(32 * h, 0),
            )
        bcs = xpool.tile([HD, 2 * 512], F32, name="bcs")
        nc.vector.reciprocal_approx_fast(bcs[:], bc[0:HD, :])
        nc.vector.tensor_mul(
            xt2[c][ds(ho, HD), :], xp[0:HD, :], bcs[:]
        )

    import os

    if os.environ.get("KDBG", "0") == "1":
        dbg = [
            ("kvT0", kvT[0][:], [P, 2 * S]), ("kvT1", kvT[1][:], [P, 2 * S]),
            ("qT0", qT[0][:], [P, S]), ("qT1", qT[1][:], [P, S]),
            ("xt20", xt2[0][:], [P, S]), ("xt21", xt2[1][:], [P, S]),
            ("TE", TE[:], [P, S]), ("kvta", kvta[:], [P, HD + 1]),
            ("rec", rec[:], [P, S]),
        ]
        for nm, ap, shp in dbg:
            dt_ = nc.dram_tensor(f"dbg_{nm}", shp, BF16, kind="ExternalOutput").ap()
            nc.sync.dma_start(out=dt_, in_=ap)

    # ---- output projection: out[s, e] = sum_c xt2[c].T @ wo[c] ----
    for st in range(8):
        pf = psA.tile([P, 2 * 512], F32, name="pf", tag="psA")
        for c in range(2):
            nc.tensor.matmul(
                pf[:, 0:E],
                xt2[c][:, ts(st, P)],
                wo[:, c, :],
                start=(c == 0), stop=(c == 1),
            )
        ob = outp.tile([P, E], F32, name="ob")
        nc.vector.tensor_copy(ob[:], pf[:, 0:E])
        nc.sync.dma_start(out=out[ts(st, P), :], in_=ob[:])


_NC_CACHE = None


def build_nc():
    global _NC_CACHE
    if _NC_CACHE is None:
        nc = bacc.Bacc(
            "TRN2",
            target_bir_lowering=False,
            debug=False,
            num_devices=8,
        )
        with tile.TileContext(nc) as tc, ExitStack() as ctx:
            _build_body(ctx, tc)
        nc.compile()
        _NC_CACHE = nc
    return _NC_CACHE


def make_in_maps(l2r_embed, r2l_embed, temb, W_dense, W_out):
    bf16 = mybir.dt.np(BF16)
    in_maps = []
    for core in range(8):
        b, hg = core // 2, core % 2
        cols = slice(CS * hg, CS * (hg + 1))
        tmb = np.zeros((D, 2), np.float16)
        tmb[:, 0] = temb[b].astype(np.float16)
        in_maps.append(
            {
                "xlT": np.ascontiguousarray(l2r_embed[b].T).astype(np.float16),
                "xrT": np.ascontiguousarray(r2l_embed[b].T).astype(np.float16),
                "tembT": tmb,
                "wkT": np.ascontiguousarray(W_dense[cols, :].T).astype(np.float16),
                "woT": np.ascontiguousarray(W_out[:, cols].T).astype(bf16),
            }
        )
    return in_maps


def kernel(l2r_embed, r2l_embed, temb, W_dense, W_out, b_out, num_heads, **run_kwargs):
    assert int(num_heads) == 8
    l2r_embed = np.asarray(l2r_embed, np.float32)
    r2l_embed = np.asarray(r2l_embed, np.float32)
    temb = np.asarray(temb, np.float32)
    W_dense = np.asarray(W_dense, np.float32)
    W_out = np.asarray(W_out, np.float32)
    b_out = np.asarray(b_out, np.float32)

    nc = build_nc()
    in_maps = make_in_maps(l2r_embed, r2l_embed, temb, W_dense, W_out)
    res = run_bass_kernel_spmd(nc, in_maps, core_ids=list(range(8)), **run_kwargs)

    B = l2r_embed.shape[0]
    outp = np.empty((B, S, E), np.float32)
    for b in range(B):
        outp[b] = (
            res.results[2 * b]["out_part"]
            + res.results[2 * b + 1]["out_part"]
            + b_out[None, :]
        )
    if run_kwargs:
        kernel.last_results = res
    return outp
